# revision 1
# baseline (speedup 1.0000x reference)
"""Masked causal attention (B=2, T=2048, C=1024, N=16 heads, D=64) on 8 TRN2 cores.

Sharding: tensor-parallel over heads. Core c computes heads 2c, 2c+1 (a
contiguous 128-channel block) for both batches: Q/K/V projections for its
head block, causal-masked softmax attention, and its partial contribution
to the output projection (Wo rows for its channels). The host sums the 8
partial outputs and adds bo.

Per-core dataflow (fp32r matmuls; exp output + V in bf16; contraction
always on partitions):
  srcT [C, M]  -> Q^T, K^T, V^T [128, T] per batch via lhsT=W-slices
  V^T -> V [s, 128] via PE identity transpose (keeps every big matmul's
  moving dim >= 256, which is what holds fp32r at 1 cycle/row)
  scores^T [s, t] = (K^T slice).T @ Q^T chunk   (contraction over d=64)
  causal handling: fully-masked column ranges of a diagonal block are
  simply not computed (matmul/exp run on the surviving columns only); only
  the 128-wide straddling sub-block gets an additive -3e4 mask
  attn_out^T and the softmax denominator come from one matmul per s-tile:
  lhsT = [v_head | ones] (65 cols) -> psum rows 0..63 = unnorm out^T,
  row 64 = sum of exp.  Normalize with reciprocal + partition_broadcast.
  o-proj: psum[t,c] = aoT.T @ Wo_slice, written to DRAM as fp32 partial.

All tensors that attention reads are split per batch so the Tile scheduler
can overlap batch-0 attention with batch-1 projections, and batch-0 output
projection with batch-1 attention.
"""

import sys

sys.path.insert(0, "/opt/trn_rl_repo")

import numpy as np

B, T, C = 2, 2048, 1024
NHEADS = 16
D = 64
M = B * T          # 4096 flattened rows
P = 128            # partitions
KC = C // P        # 8 contraction tiles
TC = 512           # t-chunk (matmul free dim)
NMC = T // TC      # 4 m-chunks per batch
NST = T // P       # 16 s-tiles per batch
NTC = T // TC      # 4 t-chunks per batch
MASK_NEG = -30000.0

_CACHE = {}


def _build_program(repeat=1):
    import concourse.bass as bass
    from concourse import bacc
    import concourse.mybir as mybir
    from concourse.tile import TileContext

    dt = mybir.dt
    nc = bacc.Bacc("TRN2", target_bir_lowering=False, debug=False, num_devices=8)

    srct = nc.dram_tensor("srct", [B * NMC, P, KC, TC], dt.float32r, kind="ExternalInput")
    wqkv = nc.dram_tensor("wqkv", [C, 3 * P], dt.float32r, kind="ExternalInput")
    wo = nc.dram_tensor("wo", [P, C], dt.float32r, kind="ExternalInput")
    bias = nc.dram_tensor("bias", [P, 3], dt.float32, kind="ExternalInput")
    m0 = nc.dram_tensor("m0", [P, P], dt.float32, kind="ExternalInput")
    ident = nc.dram_tensor("ident", [P, P], dt.float32r, kind="ExternalInput")
    ones = nc.dram_tensor("ones", [P, NST], dt.bfloat16, kind="ExternalInput")
    out = nc.dram_tensor("out", [M, C], dt.float32, kind="ExternalOutput")

    wqkv_t = wqkv.ap().rearrange("(ko p) j -> p ko j", p=P)

    ACT_ID = mybir.ActivationFunctionType.Identity
    ACT_EXP = mybir.ActivationFunctionType.Exp

    with TileContext(nc) as tc:
        with (
            tc.tile_pool(name="persist", bufs=1) as persist,
            tc.tile_pool(name="srcp", bufs=4) as srcp,
            tc.tile_pool(name="vtp", bufs=2) as vtp,
            tc.tile_pool(name="ep", bufs=6) as ep,
            tc.tile_pool(name="nrm", bufs=3) as nrm,
            tc.tile_pool(name="outp", bufs=4) as outp,
            tc.tile_pool(name="psj", bufs=2, space="PSUM") as psj,
            tc.tile_pool(name="pss", bufs=2, space="PSUM") as pss,
            tc.tile_pool(name="pso", bufs=2, space="PSUM") as pso,
            tc.tile_pool(name="psp", bufs=2, space="PSUM") as psp,
        ):
            # ---- persistent SBUF tensors; first-use order matters for the
            # serial SP DMA-issue stream: src chunk 0 + wqkv gate the first
            # matmul, everything else is needed later ----
            src0_sb = srcp.tile([P, KC, TC], dt.float32r, name="src_sb",
                                tag="src_sb")
            nc.sync.dma_start(out=src0_sb[:], in_=srct.ap()[0])
            wqkv_sb = persist.tile([P, KC, 3 * P], dt.float32r, name="wqkv_sb")
            nc.sync.dma_start(out=wqkv_sb[:], in_=wqkv_t)
            bias_sb = persist.tile([P, 3], dt.float32, name="bias_sb")
            nc.sync.dma_start(out=bias_sb[:], in_=bias.ap())
            ident_sb = persist.tile([P, P], dt.float32r, name="ident_sb")
            nc.sync.dma_start(out=ident_sb[:], in_=ident.ap())
            m0_sb = persist.tile([P, P], dt.float32, name="m0_sb")
            wo_sb = persist.tile([P, C], dt.float32r, name="wo_sb")

            # per-batch activation tensors (split so deps stay per-batch)
            qT_sb, kT_sb, v_sb, aoT_sb = [], [], [], []
            for b in range(B):
                qT_sb.append(persist.tile([P, T], dt.float32r, name=f"qT{b}_sb"))
                kT_sb.append(persist.tile([P, T], dt.float32r, name=f"kT{b}_sb"))
                # v layout per s-tile: [vA(0:64)|ones(64)|vB(65:129)|ones(129)]
                vb = persist.tile([P, NST, 130], dt.bfloat16, name=f"v{b}_sb")
                v_sb.append(vb)
                aoT_sb.append(persist.tile([P, T], dt.float32r, name=f"ao{b}_sb"))

            def emit_deferred_loads():
                for b in range(B):
                    nc.sync.dma_start(out=v_sb[b][:, :, 64], in_=ones.ap())
                    nc.sync.dma_start(out=v_sb[b][:, :, 129], in_=ones.ap())
                nc.sync.dma_start(out=m0_sb[:], in_=m0.ap())
                nc.sync.dma_start(out=wo_sb[:], in_=wo.ap())

            def emit_proj(b, first_src=None):
                for mc in range(NMC):
                    msl = slice(mc * TC, (mc + 1) * TC)
                    gsl = slice(b * T + mc * TC, b * T + (mc + 1) * TC)
                    if mc == 0 and first_src is not None:
                        src_sb = first_src
                    else:
                        src_sb = srcp.tile([P, KC, TC], dt.float32r,
                                           name="src_sb", tag="src_sb")
                        nc.sync.dma_start(out=src_sb[:],
                                          in_=srct.ap()[b * NMC + mc])

                    ps_q = psj.tile([P, TC], dt.float32, name="ps_q", tag="psj")
                    for ko in range(KC):
                        nc.tensor.matmul(
                            ps_q[:], wqkv_sb[:, ko, 0:P], src_sb[:, ko, :],
                            start=(ko == 0), stop=(ko == KC - 1),
                        )
                    nc.vector.tensor_scalar(
                        qT_sb[b][:, msl], ps_q[:], 0.125, bias_sb[:, 0:1],
                        mybir.AluOpType.mult, mybir.AluOpType.add,
                    )

                    ps_k = psj.tile([P, TC], dt.float32, name="ps_k", tag="psj")
                    for ko in range(KC):
                        nc.tensor.matmul(
                            ps_k[:], wqkv_sb[:, ko, P:2 * P], src_sb[:, ko, :],
                            start=(ko == 0), stop=(ko == KC - 1),
                        )
                    nc.vector.tensor_scalar(
                        kT_sb[b][:, msl], ps_k[:], bias_sb[:, 1:2], None,
                        mybir.AluOpType.add,
                    )

                    ps_v = psj.tile([P, TC], dt.float32, name="ps_v", tag="psj")
                    for ko in range(KC):
                        nc.tensor.matmul(
                            ps_v[:], wqkv_sb[:, ko, 2 * P:3 * P], src_sb[:, ko, :],
                            start=(ko == 0), stop=(ko == KC - 1),
                        )
                    vt_sb = vtp.tile([P, TC], dt.float32r, name="vt_sb")
                    nc.vector.tensor_scalar(
                        vt_sb[:], ps_v[:], bias_sb[:, 2:3], None,
                        mybir.AluOpType.add,
                    )
                    # transpose the 4 [128,128] blocks of this chunk into v_sb
                    for k4 in range(TC // P):
                        st = mc * (TC // P) + k4
                        ps_t = psp.tile([P, P], dt.float32r, name="ps_t", tag="psp")
                        nc.tensor.transpose(
                            ps_t[:], vt_sb[:, k4 * P:(k4 + 1) * P], ident_sb[:]
                        )
                        nc.vector.tensor_copy(v_sb[b][:, st, 0:64],
                                              ps_t[:, 0:64])
                        nc.vector.tensor_copy(v_sb[b][:, st, 65:129],
                                              ps_t[:, 64:128])

            def emit_attn(b, tcs=None):
                for tci in (tcs if tcs is not None else range(NTC)):
                    t0 = tci * TC
                    n_st = (tci + 1) * (TC // P)
                    for h in range(2):
                        jh = h * 64
                        vcol = h * 65
                        ps_o = pso.tile([P, TC], dt.float32, name="ps_o", tag="pso")
                        for st in range(n_st):
                            s0 = st * P
                            k = st - 4 * tci  # >=0 on diagonal blocks
                            toff = max(0, k) * P     # first surviving column
                            L = TC - toff            # surviving width
                            ps_s = pss.tile([P, TC], dt.float32, name="ps_s",
                                            tag="pss")
                            nc.tensor.matmul(
                                ps_s[:, 0:L],
                                kT_sb[b][jh:jh + 64, s0:s0 + P],
                                qT_sb[b][jh:jh + 64, t0 + toff:t0 + TC],
                                start=True, stop=True,
                            )
                            if k >= 0:  # straddling sub-block: mask t<s part
                                nc.vector.tensor_tensor(
                                    ps_s[:, 0:P], ps_s[:, 0:P], m0_sb[:],
                                    mybir.AluOpType.add,
                                )
                            e_sb = ep.tile([P, TC], dt.bfloat16, name="e_sb")
                            nc.scalar.activation(e_sb[:, 0:L], ps_s[:, 0:L],
                                                 ACT_EXP)
                            nc.tensor.matmul(
                                ps_o[0:65, toff:TC],
                                v_sb[b][:, st, vcol:vcol + 65],
                                e_sb[:, 0:L],
                                start=(st == 0), stop=(st == n_st - 1),
                            )
                        # normalize rows 0..63 by row 64
                        rc_sb = nrm.tile([65, TC], dt.float32, name="rc_sb")
                        nc.vector.reciprocal(rc_sb[64:65, :], ps_o[64:65, :])
                        rc0_sb = nrm.tile([1, TC], dt.float32, name="rc0_sb")
                        nc.gpsimd.dma_start(out=rc0_sb[:], in_=rc_sb[64:65, :])
                        rb_sb = nrm.tile([64, TC], dt.float32, name="rb_sb")
                        nc.gpsimd.partition_broadcast(rb_sb[:], rc0_sb[:])
                        nc.vector.tensor_tensor(
                            aoT_sb[b][jh:jh + 64, t0:t0 + TC],
                            ps_o[0:64, :], rb_sb[:],
                            mybir.AluOpType.mult,
                        )

            def emit_oproj(b, mts=None):
                for mt in (mts if mts is not None else range(T // P)):
                    for cc in range(C // TC):
                        ps_p = psp.tile([P, TC], dt.float32, name="ps_p", tag="psp")
                        nc.tensor.matmul(
                            ps_p[:],
                            aoT_sb[b][:, mt * P:(mt + 1) * P],
                            wo_sb[:, cc * TC:(cc + 1) * TC],
                            start=True, stop=True,
                        )
                        o_sb = outp.tile([P, TC], dt.float32, name="o_sb")
                        nc.vector.tensor_copy(o_sb[:], ps_p[:])
                        nc.sync.dma_start(
                            out=out.ap()[b * T + mt * P:b * T + (mt + 1) * P,
                                         cc * TC:(cc + 1) * TC],
                            in_=o_sb[:],
                        )

            first = src0_sb
            for _ in range(repeat):
                # emission order = scheduler priority: latency-critical
                # attention chains first, slack work (next batch's
                # projections, output projections) after, as gap filler
                emit_proj(0, first_src=first)
                first = None
                emit_deferred_loads()
                emit_attn(0)
                emit_proj(1)
                emit_attn(1)
                emit_oproj(0)
                emit_oproj(1)

    nc.compile()
    return nc


def _host_inputs(src, mask, Wq, bq, Wk, bk, Wv, bv, Wo, bo):
    f32 = np.float32
    src = np.asarray(src, f32)
    # [B*NMC, P, KC, TC]: per-chunk contiguous srcT tiles (k-tile-major rows)
    srct = np.ascontiguousarray(
        src.reshape(M, C).T.reshape(KC, P, B * NMC, TC).transpose(2, 1, 0, 3)
    )

    # straddle mask: m0[p, f] = 0 (keep) iff f >= p, else MASK_NEG
    f = np.arange(P)[None, :]
    s = np.arange(P)[:, None]
    m0 = np.where(f >= s, 0.0, MASK_NEG).astype(f32)
    ident = np.eye(P, dtype=f32)

    in_maps = []
    for c in range(8):
        sl = slice(c * P, (c + 1) * P)
        wqkv = np.concatenate(
            [np.asarray(Wq, f32)[:, sl], np.asarray(Wk, f32)[:, sl],
             np.asarray(Wv, f32)[:, sl]], axis=1,
        )
        bias = np.stack(
            [np.asarray(bq, f32)[sl] * 0.125, np.asarray(bk, f32)[sl],
             np.asarray(bv, f32)[sl]], axis=1,
        ).astype(f32)
        wo_c = np.ascontiguousarray(np.asarray(Wo, f32)[sl, :])
        in_maps.append({
            "srct": srct, "wqkv": np.ascontiguousarray(wqkv), "wo": wo_c,
            "bias": np.ascontiguousarray(bias), "m0": m0, "ident": ident,
            "ones": np.ones((P, NST), __import__("ml_dtypes").bfloat16),
        })
    return in_maps


def kernel(src, mask, Wq, bq, Wk, bk, Wv, bv, Wo, bo):
    from concourse.bass_utils import run_bass_kernel_spmd

    if "nc" not in _CACHE:
        _CACHE["nc"] = _build_program()
    nc = _CACHE["nc"]

    in_maps = _host_inputs(src, mask, Wq, bq, Wk, bk, Wv, bv, Wo, bo)
    res = run_bass_kernel_spmd(nc, in_maps, list(range(8)))

    acc = np.zeros((M, C), np.float64)
    for c in range(8):
        acc += res.results[c]["out"]
    acc += np.asarray(bo, np.float64)[None, :]
    return acc.astype(np.float32).reshape(B, T, C)

